# revision 32
# baseline (speedup 1.0000x reference)
"""Trainium2 Bass kernel for nn_BilinearAttentionFusion.

Math (see reference):
    b_mean = mean_j feat_b[b, j, :]                      [32, 512]
    t[b, k, d] = sum_e W[k, d, e] * b_mean[b, e]         [32, 512, 512]
    fused = feat_a @ t^T + bias                          [32, 300, 512]
    out = LayerNorm(fused + feat_a) * gamma + beta

Distribution (8 NeuronCores, 3 SPMD launches, no collectives —
collectives cost 60-170us of cross-core sync under this runtime):
    K1 (j-sharded): core i reduces feat_b[:, 128i:128(i+1), :] to a
        partial b_meanT [e, b] (scaled 1/1024). Host sums the 8 partials.
    K2 (k-sharded): core i owns W[64i:64(i+1)], host-transposed to
        [e, (d, k_loc)] and cast to fp16 (the 512 MB W stream is the
        HBM roofline term; fp16 halves it, rel-err ~1e-3 << 2e-2 tol).
        Streams W through the PE as the moving operand vs the tiny
        stationary b_meanT -> t_shard fp16. Pure per-core streaming.
    host: concat t shards over k -> t[b, d, k], add I (so the residual
        x = fused + feat_a comes out of the K3 matmul directly), concat
        with feat_aT along the free axis, reshard by batch.
    K3 (batch-sharded): core j owns batches 4j..4j+3:
        x[b] = feat_aT[b]^T @ (t[b] + I) + bias  (contract d, fp16 PE),
        LayerNorm on DVE+ACT, gamma/beta (skipped when ones/zeros).

Matmuls accumulate fp32 in PSUM; only the W/t/feat_a matmul operands
are fp16.
"""
import sys

for _p in ("/opt/trn_rl_repo", "/root/.axon_site", "/root/.axon_site/_ro/pypackages"):
    if _p not in sys.path:
        sys.path.append(_p)

import numpy as np
import concourse.bacc as bacc
import concourse.tile as tile
from concourse import mybir
from concourse.bass_utils import run_bass_kernel_spmd

N_CORES = 8
BS, LEN_A, LEN_B, H = 32, 300, 1024, 512
K_SH = H // N_CORES  # 64 k-columns of W per core in K2
B_SH = BS // N_CORES  # 4 batches per core in K3
J_SH = LEN_B // N_CORES  # 128 j-rows of feat_b per core in K1
LN_EPS = 1e-5

F32 = mybir.dt.float32
F16 = mybir.dt.float16
F8 = mybir.dt.float8e3  # e3m4: 4 mantissa bits, range +-15.5 — fits N(0,1) W

DK = H * K_SH  # 32768 flattened (d, k_loc) columns per core in K2
WCOLS = 4096  # K2 W-streaming tile free size (1 MiB fp16 tiles)
ET = H // 128  # 4 contraction e-tiles
A_TILES = [(0, 128), (128, 128), (256, 44)]  # len_a = 300
MW = H + LEN_A  # 812: K3 per-row concat of (t+I | feat_aT)


def _build_k1():
    nc = bacc.Bacc(trn_type="TRN2", num_devices=N_CORES)
    # flat (b, j) free axis: per-partition reads are one contiguous 8 KB
    # span per tile (the 3D [e, b, j] AP emitted 16x 512 B descriptors)
    fbt = nc.dram_tensor("fbt", [H, BS * J_SH], F32, kind="ExternalInput")
    # partition-major out layout: one contiguous 512 B span per partition
    # (host untangles [p, et, b] -> [e, b] for free)
    pb_out = nc.dram_tensor("pb", [128, ET * BS], F32, kind="ExternalOutput")
    HALF = BS // 2 * J_SH  # 2048 cols per half-tile
    with tile.TileContext(nc) as tc:
        with (
            tc.tile_pool(name="fb", bufs=8) as fbp,
            tc.tile_pool(name="small", bufs=1) as small,
        ):
            # single batched pb tile -> one [128, ET*BS] out-DMA at the end
            # (4 separate [128, 32] writes were 128 B/partition descriptors
            # at ~20 GB/s, ~3 us of tail)
            pb = small.tile([128, ET, BS], F32)
            for et in range(ET):
                # quarter-tiles on the last et shorten the trailing reduce
                nh = 4 if et == ET - 1 else 2
                step = BS // nh
                for h in range(nh):
                    bs0 = h * step
                    fb_t = fbp.tile([128, HALF], F32, tag="fb")
                    nc.sync.dma_start(
                        out=fb_t[: , : step * J_SH],
                        in_=fbt[
                            et * 128 : (et + 1) * 128,
                            bs0 * J_SH : (bs0 + step) * J_SH,
                        ],
                    )
                    nc.vector.reduce_sum(
                        out=pb[:, et, bs0 : bs0 + step],
                        in_=fb_t[:, : step * J_SH].rearrange(
                            "p (b j) -> p b j", j=J_SH
                        ),
                        axis=mybir.AxisListType.X,
                    )
            nc.scalar.mul(out=pb[:], in_=pb[:], mul=1.0 / LEN_B)
            nc.scalar.dma_start(out=pb_out.ap(), in_=pb[:])
    nc.finalize()
    return nc


def _build_k2():
    nc = bacc.Bacc(trn_type="TRN2", num_devices=N_CORES)
    bm = nc.dram_tensor("bm", [H, BS], F16, kind="ExternalInput")
    # W streams as fp8 e3m4 (1 B/elem): measured end-to-end rel_rms 1.15e-2
    # vs the 2e-2 gate (fp16 was 3.8e-4). Halves the HBM-stack-pair floor
    # for the W read from 93.8 us to 46.9 us. bm stays fp16 (stationary).
    wt = nc.dram_tensor("wt", [H, DK], F8, kind="ExternalInput")
    # b-major layout: stage writes are one contiguous 8 KB span per
    # partition (chunk-major emitted 1 KB descriptors at ~66 GB/s which
    # also stole SDMA service slots from the W in-stream)
    t_out = nc.dram_tensor("t_out", [BS, DK], F16, kind="ExternalOutput")

    with tile.TileContext(nc) as tc:
        with (
            tc.tile_pool(name="bm", bufs=1) as bmp,
            tc.tile_pool(name="wtiles", bufs=12) as wp,
            tc.tile_pool(name="ps", bufs=8, space="PSUM") as ps,
            tc.tile_pool(name="tstage", bufs=3) as tsp,
        ):
            bmt = bmp.tile([128, ET, BS], F16)
            nc.sync.dma_start(out=bmt[:], in_=bm.ap().rearrange("(t p) b -> p t b", p=128))

            # taper the final groups so the trailing PE work after the last
            # W DMA (which nothing overlaps) is small
            groups = [(gi * WCOLS, WCOLS) for gi in range(DK // WCOLS - 1)]
            last = DK - WCOLS
            groups += [(last, 2048), (last + 2048, 1024), (last + 3072, 512), (last + 3584, 512)]
            for col0, width in groups:
                nchunk = width // 512
                wts = []
                for et in range(ET):
                    w_t = wp.tile([128, WCOLS], F8, tag="wt")
                    nc.sync.dma_start(
                        out=w_t[:, :width],
                        in_=wt[et * 128 : (et + 1) * 128, col0 : col0 + width],
                    )
                    wts.append(w_t)
                psums = [
                    ps.tile([BS, 512], F32, tag="psum", name=f"psum{c}")
                    for c in range(nchunk)
                ]
                for et in range(ET):
                    for c in range(nchunk):
                        nc.tensor.matmul(
                            out=psums[c][:],
                            lhsT=bmt[:, et, :],
                            rhs=wts[et][:, c * 512 : (c + 1) * 512],
                            start=(et == 0),
                            stop=(et == ET - 1),
                        )
                # casts alternate DVE/ACT: PE is the pacer now, and a lagging
                # cast queue stalls psum-bank reuse (DVE was 50% busy on
                # casts alone); single per-group out-DMA is kept
                stage = tsp.tile([BS, WCOLS // 512, 512], F16, tag="stage")
                for c in range(nchunk):
                    if c % 2 == 0:
                        nc.vector.tensor_copy(stage[:, c, :], psums[c][:])
                    else:
                        nc.scalar.activation(
                            out=stage[:, c, :],
                            in_=psums[c][:],
                            func=mybir.ActivationFunctionType.Copy,
                        )
                nc.scalar.dma_start(
                    out=t_out[:, col0 : col0 + width],
                    in_=stage[:, :nchunk, :],
                )
    nc.finalize()
    return nc


def _build_k3(apply_affine):
    nc = bacc.Bacc(trn_type="TRN2", num_devices=N_CORES)
    # m[b] = [512(d), 512(k) of t+I | 300(a) of feat_aT], all fp16
    m = nc.dram_tensor("m", [B_SH, H, MW], F16, kind="ExternalInput")
    bias_d = nc.dram_tensor("bias", [1, H], F16, kind="ExternalInput")
    gamma_d = nc.dram_tensor("gamma", [H], F32, kind="ExternalInput")
    beta_d = nc.dram_tensor("beta", [H], F32, kind="ExternalInput")
    out = nc.dram_tensor("out", [B_SH, LEN_A, H], F32, kind="ExternalOutput")

    with tile.TileContext(nc) as tc:
        with (
            tc.tile_pool(name="consts", bufs=1) as consts,
            tc.tile_pool(name="ins", bufs=4) as ins,
            tc.tile_pool(name="ps", bufs=8, space="PSUM") as ps,
            tc.tile_pool(name="work", bufs=4) as work,
            tc.tile_pool(name="small", bufs=8) as small,
        ):
            gamma_t = beta_t = None
            if apply_affine:
                gamma_t = consts.tile([128, H], F32)
                nc.sync.dma_start(
                    out=gamma_t[:], in_=gamma_d.ap().partition_broadcast(128)
                )
                beta_t = consts.tile([128, H], F32)
                nc.sync.dma_start(
                    out=beta_t[:], in_=beta_d.ap().partition_broadcast(128)
                )
            eps_t = consts.tile([128, 1], F32)
            nc.vector.memset(eps_t[:], LN_EPS)
            # bias folded into each accumulation group as a K=1 matmul:
            # ones[1, aw].T @ bias16[1, 512] broadcasts bias into psum
            ones_t = consts.tile([1, 128], F16)
            nc.vector.memset(ones_t[:], 1.0)
            bias_t = consts.tile([1, H], F16)
            nc.sync.dma_start(out=bias_t[:], in_=bias_d.ap())

            gi = 0
            for b in range(B_SH):
                # one 832 KB DMA per batch: issue cost (~620 ns each on the
                # sync engine) was serializing 16 small loads
                m_t = ins.tile([128, ET, MW], F16, tag="m")
                nc.sync.dma_start(
                    out=m_t[:],
                    in_=m.ap()[b].rearrange("(t p) w -> p t w", p=128),
                )
                for a0, aw in A_TILES:
                    gi += 1
                    psum = ps.tile([aw, H], F32, tag="psum")
                    nc.tensor.matmul(
                        out=psum[:],
                        lhsT=ones_t[:, :aw],
                        rhs=bias_t[:],
                        start=True,
                        stop=False,
                    )
                    for dt_i in range(ET):
                        nc.tensor.matmul(
                            out=psum[:],
                            lhsT=m_t[:, dt_i, H + a0 : H + a0 + aw],
                            rhs=m_t[:, dt_i, 0:H],
                            start=False,
                            stop=(dt_i == ET - 1),
                        )
                    # x = psum holds fused + residual + bias; LN reads PSUM.
                    # Short chain (2 engine hops): DVE stats/aggr -> ACT
                    # rsqrt -> DVE normalize. The 6-op/5-hop version paced
                    # the whole kernel at ~2.5 us per tile.
                    stats = small.tile([aw, 6], F32, tag="stats")
                    nc.vector.bn_stats(out=stats[:], in_=psum[:])
                    mv = small.tile([aw, 2], F32, tag="mv")
                    nc.vector.bn_aggr(out=mv[:], in_=stats[:])
                    rstd = small.tile([aw, 1], F32, tag="rstd")
                    nc.scalar.activation(
                        out=rstd[:],
                        in_=mv[:, 1:2],
                        func=mybir.ActivationFunctionType.Sqrt,
                        bias=eps_t[:aw, :],
                        scale=1.0,
                    )
                    nc.vector.reciprocal(out=rstd[:], in_=rstd[:])
                    xn = work.tile([aw, H], F32, tag="xn")
                    if gi % 2 == 0:
                        # DVE normalize: (x - mu) * rstd in one op
                        nc.vector.tensor_scalar(
                            out=xn[:],
                            in0=psum[:],
                            scalar1=mv[:, 0:1],
                            scalar2=rstd[:],
                            op0=mybir.AluOpType.subtract,
                            op1=mybir.AluOpType.mult,
                        )
                    else:
                        # ACT normalize: x*rstd + (-mu*rstd); alternating
                        # engines lets the LN pipeline pace at the DVE's
                        # ~1.4 us/group instead of 1.8
                        nmr = small.tile([aw, 1], F32, tag="nmr")
                        nc.vector.tensor_scalar(
                            out=nmr[:],
                            in0=mv[:, 0:1],
                            scalar1=rstd[:],
                            scalar2=-1.0,
                            op0=mybir.AluOpType.mult,
                            op1=mybir.AluOpType.mult,
                        )
                        nc.scalar.activation(
                            out=xn[:],
                            in_=psum[:],
                            func=mybir.ActivationFunctionType.Identity,
                            bias=nmr[:],
                            scale=rstd[:],
                        )
                    if apply_affine:
                        nc.vector.tensor_mul(out=xn[:], in0=xn[:], in1=gamma_t[:aw, :])
                        nc.vector.tensor_add(out=xn[:], in0=xn[:], in1=beta_t[:aw, :])
                    # out-DMAs issue from the otherwise-idle gpsimd queue
                    # so they never stall the sync/scalar critical path
                    nc.gpsimd.dma_start(out=out[b, a0 : a0 + aw, :], in_=xn[:])
    nc.finalize()
    return nc


_CACHE = {}


def _program(name, builder):
    if name not in _CACHE:
        _CACHE[name] = builder()
    return _CACHE[name]


def kernel(feat_a, feat_b, W, bias, gamma, beta, _trace=False, _timings=None):
    feat_a = np.ascontiguousarray(feat_a, dtype=np.float32)
    feat_b = np.ascontiguousarray(feat_b, dtype=np.float32)
    W = np.ascontiguousarray(W, dtype=np.float32)
    bias = np.ascontiguousarray(bias, dtype=np.float32)
    gamma = np.ascontiguousarray(gamma, dtype=np.float32)
    beta = np.ascontiguousarray(beta, dtype=np.float32)

    core_ids = list(range(N_CORES))
    affine = not (np.all(gamma == 1.0) and np.all(beta == 0.0))
    nc1 = _program("k1", _build_k1)
    nc2 = _program("k2", _build_k2)
    nc3 = _program(("k3", affine), lambda: _build_k3(affine))
    trace_kw = dict(trace=True, trace_cores=[0]) if _trace else {}

    # ---- K1: partial b_mean over j-shards ----
    in_maps1 = [
        {
            "fbt": np.ascontiguousarray(
                feat_b[:, i * J_SH : (i + 1) * J_SH, :].transpose(2, 0, 1)
            ).reshape(H, BS * J_SH)
        }
        for i in range(N_CORES)
    ]
    res1 = run_bass_kernel_spmd(nc1, in_maps1, core_ids, **trace_kw)
    if _timings is not None:
        _timings.append(res1.exec_time_ns)
    bmT = np.sum([res1.results[i]["pb"] for i in range(N_CORES)], axis=0)
    # [p, et, b] -> [e = et*128+p, b]
    bmT = bmT.reshape(128, ET, BS).transpose(1, 0, 2).reshape(H, BS)
    bmT16 = bmT.astype(np.float16)

    # ---- K2: t = W x b_mean, k-sharded fp8 W stream ----
    import ml_dtypes

    in_maps2 = []
    for i in range(N_CORES):
        wi = (
            np.ascontiguousarray(W[i * K_SH : (i + 1) * K_SH].transpose(2, 1, 0))
            .reshape(H, DK)
            .astype(ml_dtypes.float8_e3m4)
        )
        in_maps2.append({"bm": bmT16, "wt": wi})
    res2 = run_bass_kernel_spmd(nc2, in_maps2, core_ids, **trace_kw)
    if _timings is not None:
        _timings.append(res2.exec_time_ns)
    t_full = np.concatenate(
        [
            # [b, (d, k_loc)] -> [b, d, k_loc]
            res2.results[i]["t_out"].reshape(BS, H, K_SH)
            for i in range(N_CORES)
        ],
        axis=2,
    )
    # residual folded into the matmul: x = feat_a @ (t^T + I)
    di = np.arange(H)
    t_full[:, di, di] += np.float16(1.0)

    # ---- K3: fused matmul + residual + LayerNorm, batch-sharded ----
    fa16t = feat_a.astype(np.float16).transpose(0, 2, 1)  # [bs, d, a]
    bias16 = bias.astype(np.float16).reshape(1, H)
    in_maps3 = []
    for j in range(N_CORES):
        bsl = slice(j * B_SH, (j + 1) * B_SH)
        in_maps3.append(
            {
                "m": np.ascontiguousarray(
                    np.concatenate([t_full[bsl], fa16t[bsl]], axis=2)
                ),
                "bias": bias16,
                "gamma": gamma,
                "beta": beta,
            }
        )
    res3 = run_bass_kernel_spmd(nc3, in_maps3, core_ids, **trace_kw)
    if _timings is not None:
        _timings.append(res3.exec_time_ns)

    return np.concatenate([res3.results[j]["out"] for j in range(N_CORES)], axis=0)


# revision 33
# speedup vs baseline: 1.1557x; 1.1557x over previous
"""Trainium2 Bass kernel for nn_BilinearAttentionFusion.

Math (see reference):
    b_mean = mean_j feat_b[b, j, :]                      [32, 512]
    t[b, k, d] = sum_e W[k, d, e] * b_mean[b, e]         [32, 512, 512]
    fused = feat_a @ t^T + bias                          [32, 300, 512]
    out = LayerNorm(fused + feat_a) * gamma + beta

Distribution (8 NeuronCores, 3 SPMD launches, no collectives —
collectives cost 60-170us of cross-core sync under this runtime):
    K1 (j-sharded): core i reduces feat_b[:, 128i:128(i+1), :] to a
        partial b_meanT [e, b] (scaled 1/1024). Host sums the 8 partials.
    K2 (k-sharded): core i owns W[64i:64(i+1)], host-transposed to
        [e, (d, k_loc)] and cast to fp16 (the 512 MB W stream is the
        HBM roofline term; fp16 halves it, rel-err ~1e-3 << 2e-2 tol).
        Streams W through the PE as the moving operand vs the tiny
        stationary b_meanT -> t_shard fp16. Pure per-core streaming.
    host: concat t shards over k -> t[b, d, k], add I (so the residual
        x = fused + feat_a comes out of the K3 matmul directly), concat
        with feat_aT along the free axis, reshard by batch.
    K3 (batch-sharded): core j owns batches 4j..4j+3:
        x[b] = feat_aT[b]^T @ (t[b] + I) + bias  (contract d, fp16 PE),
        LayerNorm on DVE+ACT, gamma/beta (skipped when ones/zeros).

Matmuls accumulate fp32 in PSUM; only the W/t/feat_a matmul operands
are fp16.
"""
import sys

for _p in ("/opt/trn_rl_repo", "/root/.axon_site", "/root/.axon_site/_ro/pypackages"):
    if _p not in sys.path:
        sys.path.append(_p)

import numpy as np
import concourse.bacc as bacc
import concourse.tile as tile
from concourse import mybir
from concourse.bass_utils import run_bass_kernel_spmd

N_CORES = 8
BS, LEN_A, LEN_B, H = 32, 300, 1024, 512
K_SH = H // N_CORES  # 64 k-columns of W per core in K2
B_SH = BS // N_CORES  # 4 batches per core in K3
J_SH = LEN_B // N_CORES  # 128 j-rows of feat_b per core in K1
LN_EPS = 1e-5

F32 = mybir.dt.float32
F16 = mybir.dt.float16
F8 = mybir.dt.float8e3  # e3m4: 4 mantissa bits, range +-15.5 — fits N(0,1) W

DK = H * K_SH  # 32768 flattened (d, k_loc) columns per core in K2
WCOLS = 4096  # K2 W-streaming tile free size (1 MiB fp16 tiles)
ET = H // 128  # 4 contraction e-tiles
A_TILES = [(0, 128), (128, 128), (256, 44)]  # len_a = 300
MW = H + LEN_A  # 812: K3 per-row concat of (t+I | feat_aT)


def _build_k1():
    nc = bacc.Bacc(trn_type="TRN2", num_devices=N_CORES)
    # flat (b, j) free axis: per-partition reads are one contiguous 8 KB
    # span per tile (the 3D [e, b, j] AP emitted 16x 512 B descriptors)
    fbt = nc.dram_tensor("fbt", [H, BS * J_SH], F32, kind="ExternalInput")
    # partition-major out layout: one contiguous 512 B span per partition
    # (host untangles [p, et, b] -> [e, b] for free)
    pb_out = nc.dram_tensor("pb", [128, ET * BS], F32, kind="ExternalOutput")
    HALF = BS // 2 * J_SH  # 2048 cols per half-tile
    with tile.TileContext(nc) as tc:
        with (
            tc.tile_pool(name="fb", bufs=8) as fbp,
            tc.tile_pool(name="small", bufs=1) as small,
        ):
            # single batched pb tile -> one [128, ET*BS] out-DMA at the end
            # (4 separate [128, 32] writes were 128 B/partition descriptors
            # at ~20 GB/s, ~3 us of tail)
            pb = small.tile([128, ET, BS], F32)
            for et in range(ET):
                # quarter-tiles on the last et shorten the trailing reduce
                nh = 4 if et == ET - 1 else 2
                step = BS // nh
                for h in range(nh):
                    bs0 = h * step
                    fb_t = fbp.tile([128, HALF], F32, tag="fb")
                    nc.sync.dma_start(
                        out=fb_t[: , : step * J_SH],
                        in_=fbt[
                            et * 128 : (et + 1) * 128,
                            bs0 * J_SH : (bs0 + step) * J_SH,
                        ],
                    )
                    nc.vector.reduce_sum(
                        out=pb[:, et, bs0 : bs0 + step],
                        in_=fb_t[:, : step * J_SH].rearrange(
                            "p (b j) -> p b j", j=J_SH
                        ),
                        axis=mybir.AxisListType.X,
                    )
            nc.scalar.mul(out=pb[:], in_=pb[:], mul=1.0 / LEN_B)
            nc.scalar.dma_start(out=pb_out.ap(), in_=pb[:])
    nc.finalize()
    return nc


def _build_k2():
    nc = bacc.Bacc(trn_type="TRN2", num_devices=N_CORES)
    bm = nc.dram_tensor("bm", [H, BS], F16, kind="ExternalInput")
    # W streams as fp8 e3m4 (1 B/elem): measured end-to-end rel_rms 1.15e-2
    # vs the 2e-2 gate (fp16 was 3.8e-4). Halves the HBM-stack-pair floor
    # for the W read from 93.8 us to 46.9 us. bm stays fp16 (stationary).
    wt = nc.dram_tensor("wt", [H, DK], F8, kind="ExternalInput")
    # b-major layout: stage writes are one contiguous 8 KB span per
    # partition (chunk-major emitted 1 KB descriptors at ~66 GB/s which
    # also stole SDMA service slots from the W in-stream)
    t_out = nc.dram_tensor("t_out", [BS, DK], F16, kind="ExternalOutput")

    with tile.TileContext(nc) as tc:
        with (
            tc.tile_pool(name="bm", bufs=1) as bmp,
            tc.tile_pool(name="wtiles", bufs=12) as wp,
            tc.tile_pool(name="ps", bufs=8, space="PSUM") as ps,
            tc.tile_pool(name="tstage", bufs=3) as tsp,
        ):
            bmt = bmp.tile([128, ET, BS], F16)
            nc.sync.dma_start(out=bmt[:], in_=bm.ap().rearrange("(t p) b -> p t b", p=128))

            # taper the final groups so the trailing PE work after the last
            # W DMA (which nothing overlaps) is small
            groups = [(gi * WCOLS, WCOLS) for gi in range(DK // WCOLS - 1)]
            last = DK - WCOLS
            groups += [(last, 2048), (last + 2048, 1024), (last + 3072, 512), (last + 3584, 512)]
            for col0, width in groups:
                nchunk = width // 512
                wts = []
                for et in range(ET):
                    w_t = wp.tile([128, WCOLS], F8, tag="wt")
                    nc.sync.dma_start(
                        out=w_t[:, :width],
                        in_=wt[et * 128 : (et + 1) * 128, col0 : col0 + width],
                    )
                    wts.append(w_t)
                psums = [
                    ps.tile([BS, 512], F32, tag="psum", name=f"psum{c}")
                    for c in range(nchunk)
                ]
                for et in range(ET):
                    for c in range(nchunk):
                        nc.tensor.matmul(
                            out=psums[c][:],
                            lhsT=bmt[:, et, :],
                            rhs=wts[et][:, c * 512 : (c + 1) * 512],
                            start=(et == 0),
                            stop=(et == ET - 1),
                        )
                stage = tsp.tile([BS, WCOLS // 512, 512], F16, tag="stage")
                for c in range(nchunk):
                    nc.vector.tensor_copy(stage[:, c, :], psums[c][:])
                nc.scalar.dma_start(
                    out=t_out[:, col0 : col0 + width],
                    in_=stage[:, :nchunk, :],
                )
    nc.finalize()
    return nc


def _build_k3(apply_affine):
    nc = bacc.Bacc(trn_type="TRN2", num_devices=N_CORES)
    # m[b] = [512(d), 512(k) of t+I | 300(a) of feat_aT], all fp16
    m = nc.dram_tensor("m", [B_SH, H, MW], F16, kind="ExternalInput")
    bias_d = nc.dram_tensor("bias", [1, H], F16, kind="ExternalInput")
    gamma_d = nc.dram_tensor("gamma", [H], F32, kind="ExternalInput")
    beta_d = nc.dram_tensor("beta", [H], F32, kind="ExternalInput")
    out = nc.dram_tensor("out", [B_SH, LEN_A, H], F32, kind="ExternalOutput")

    with tile.TileContext(nc) as tc:
        with (
            tc.tile_pool(name="consts", bufs=1) as consts,
            tc.tile_pool(name="ins", bufs=4) as ins,
            tc.tile_pool(name="ps", bufs=8, space="PSUM") as ps,
            tc.tile_pool(name="work", bufs=4) as work,
            tc.tile_pool(name="small", bufs=8) as small,
        ):
            gamma_t = beta_t = None
            if apply_affine:
                gamma_t = consts.tile([128, H], F32)
                nc.sync.dma_start(
                    out=gamma_t[:], in_=gamma_d.ap().partition_broadcast(128)
                )
                beta_t = consts.tile([128, H], F32)
                nc.sync.dma_start(
                    out=beta_t[:], in_=beta_d.ap().partition_broadcast(128)
                )
            eps_t = consts.tile([128, 1], F32)
            nc.vector.memset(eps_t[:], LN_EPS)
            # bias folded into each accumulation group as a K=1 matmul:
            # ones[1, aw].T @ bias16[1, 512] broadcasts bias into psum
            ones_t = consts.tile([1, 128], F16)
            nc.vector.memset(ones_t[:], 1.0)
            bias_t = consts.tile([1, H], F16)
            nc.sync.dma_start(out=bias_t[:], in_=bias_d.ap())

            gi = 0
            for b in range(B_SH):
                # one 832 KB DMA per batch: issue cost (~620 ns each on the
                # sync engine) was serializing 16 small loads
                m_t = ins.tile([128, ET, MW], F16, tag="m")
                nc.sync.dma_start(
                    out=m_t[:],
                    in_=m.ap()[b].rearrange("(t p) w -> p t w", p=128),
                )
                for a0, aw in A_TILES:
                    gi += 1
                    psum = ps.tile([aw, H], F32, tag="psum")
                    nc.tensor.matmul(
                        out=psum[:],
                        lhsT=ones_t[:, :aw],
                        rhs=bias_t[:],
                        start=True,
                        stop=False,
                    )
                    for dt_i in range(ET):
                        nc.tensor.matmul(
                            out=psum[:],
                            lhsT=m_t[:, dt_i, H + a0 : H + a0 + aw],
                            rhs=m_t[:, dt_i, 0:H],
                            start=False,
                            stop=(dt_i == ET - 1),
                        )
                    # x = psum holds fused + residual + bias; LN reads PSUM.
                    # Short chain (2 engine hops): DVE stats/aggr -> ACT
                    # rsqrt -> DVE normalize. The 6-op/5-hop version paced
                    # the whole kernel at ~2.5 us per tile.
                    stats = small.tile([aw, 6], F32, tag="stats")
                    nc.vector.bn_stats(out=stats[:], in_=psum[:])
                    mv = small.tile([aw, 2], F32, tag="mv")
                    nc.vector.bn_aggr(out=mv[:], in_=stats[:])
                    rstd = small.tile([aw, 1], F32, tag="rstd")
                    nc.scalar.activation(
                        out=rstd[:],
                        in_=mv[:, 1:2],
                        func=mybir.ActivationFunctionType.Sqrt,
                        bias=eps_t[:aw, :],
                        scale=1.0,
                    )
                    nc.vector.reciprocal(out=rstd[:], in_=rstd[:])
                    xn = work.tile([aw, H], F32, tag="xn")
                    if gi % 2 == 0:
                        # DVE normalize: (x - mu) * rstd in one op
                        nc.vector.tensor_scalar(
                            out=xn[:],
                            in0=psum[:],
                            scalar1=mv[:, 0:1],
                            scalar2=rstd[:],
                            op0=mybir.AluOpType.subtract,
                            op1=mybir.AluOpType.mult,
                        )
                    else:
                        # ACT normalize: x*rstd + (-mu*rstd); alternating
                        # engines lets the LN pipeline pace at the DVE's
                        # ~1.4 us/group instead of 1.8
                        nmr = small.tile([aw, 1], F32, tag="nmr")
                        nc.vector.tensor_scalar(
                            out=nmr[:],
                            in0=mv[:, 0:1],
                            scalar1=rstd[:],
                            scalar2=-1.0,
                            op0=mybir.AluOpType.mult,
                            op1=mybir.AluOpType.mult,
                        )
                        nc.scalar.activation(
                            out=xn[:],
                            in_=psum[:],
                            func=mybir.ActivationFunctionType.Identity,
                            bias=nmr[:],
                            scale=rstd[:],
                        )
                    if apply_affine:
                        nc.vector.tensor_mul(out=xn[:], in0=xn[:], in1=gamma_t[:aw, :])
                        nc.vector.tensor_add(out=xn[:], in0=xn[:], in1=beta_t[:aw, :])
                    # out-DMAs issue from the otherwise-idle gpsimd queue
                    # so they never stall the sync/scalar critical path
                    nc.gpsimd.dma_start(out=out[b, a0 : a0 + aw, :], in_=xn[:])
    nc.finalize()
    return nc


_CACHE = {}


def _program(name, builder):
    if name not in _CACHE:
        _CACHE[name] = builder()
    return _CACHE[name]


def kernel(feat_a, feat_b, W, bias, gamma, beta, _trace=False, _timings=None):
    feat_a = np.ascontiguousarray(feat_a, dtype=np.float32)
    feat_b = np.ascontiguousarray(feat_b, dtype=np.float32)
    W = np.ascontiguousarray(W, dtype=np.float32)
    bias = np.ascontiguousarray(bias, dtype=np.float32)
    gamma = np.ascontiguousarray(gamma, dtype=np.float32)
    beta = np.ascontiguousarray(beta, dtype=np.float32)

    core_ids = list(range(N_CORES))
    affine = not (np.all(gamma == 1.0) and np.all(beta == 0.0))
    nc1 = _program("k1", _build_k1)
    nc2 = _program("k2", _build_k2)
    nc3 = _program(("k3", affine), lambda: _build_k3(affine))
    trace_kw = dict(trace=True, trace_cores=[0]) if _trace else {}

    # ---- K1: partial b_mean over j-shards ----
    in_maps1 = [
        {
            "fbt": np.ascontiguousarray(
                feat_b[:, i * J_SH : (i + 1) * J_SH, :].transpose(2, 0, 1)
            ).reshape(H, BS * J_SH)
        }
        for i in range(N_CORES)
    ]
    res1 = run_bass_kernel_spmd(nc1, in_maps1, core_ids, **trace_kw)
    if _timings is not None:
        _timings.append(res1.exec_time_ns)
    bmT = np.sum([res1.results[i]["pb"] for i in range(N_CORES)], axis=0)
    # [p, et, b] -> [e = et*128+p, b]
    bmT = bmT.reshape(128, ET, BS).transpose(1, 0, 2).reshape(H, BS)
    bmT16 = bmT.astype(np.float16)

    # ---- K2: t = W x b_mean, k-sharded fp8 W stream ----
    import ml_dtypes

    in_maps2 = []
    for i in range(N_CORES):
        wi = (
            np.ascontiguousarray(W[i * K_SH : (i + 1) * K_SH].transpose(2, 1, 0))
            .reshape(H, DK)
            .astype(ml_dtypes.float8_e3m4)
        )
        in_maps2.append({"bm": bmT16, "wt": wi})
    res2 = run_bass_kernel_spmd(nc2, in_maps2, core_ids, **trace_kw)
    if _timings is not None:
        _timings.append(res2.exec_time_ns)
    t_full = np.concatenate(
        [
            # [b, (d, k_loc)] -> [b, d, k_loc]
            res2.results[i]["t_out"].reshape(BS, H, K_SH)
            for i in range(N_CORES)
        ],
        axis=2,
    )
    # residual folded into the matmul: x = feat_a @ (t^T + I)
    di = np.arange(H)
    t_full[:, di, di] += np.float16(1.0)

    # ---- K3: fused matmul + residual + LayerNorm, batch-sharded ----
    fa16t = feat_a.astype(np.float16).transpose(0, 2, 1)  # [bs, d, a]
    bias16 = bias.astype(np.float16).reshape(1, H)
    in_maps3 = []
    for j in range(N_CORES):
        bsl = slice(j * B_SH, (j + 1) * B_SH)
        in_maps3.append(
            {
                "m": np.ascontiguousarray(
                    np.concatenate([t_full[bsl], fa16t[bsl]], axis=2)
                ),
                "bias": bias16,
                "gamma": gamma,
                "beta": beta,
            }
        )
    res3 = run_bass_kernel_spmd(nc3, in_maps3, core_ids, **trace_kw)
    if _timings is not None:
        _timings.append(res3.exec_time_ns)

    return np.concatenate([res3.results[j]["out"] for j in range(N_CORES)], axis=0)
